# revision 20
# baseline (speedup 1.0000x reference)
"""Fused transformer decoder block on 8 Trainium2 NeuronCores.

Sharding: Ulysses-style. Each core computes QKV projections for its 2 heads
(x2 batches) over all tokens, runs causal attention for those heads, then an
AllToAll redistributes context from head-sharded to token-sharded; each core
finishes out-proj + LN1 + MLP + LN2 for its 512-token shard.

All activations live in transposed layout (model dim on partitions, tokens on
the free axis) so every matmul contraction runs over the partition dim with
no activation transposes. Matmuls use float32r (TF32-like, full PE rate).
"""

import sys

sys.path.insert(0, "/opt/trn_rl_repo")

import numpy as np

import concourse.bass as bass
import concourse.mybir as mybir
import concourse.tile as tile
from concourse import bacc
from concourse.bass_utils import run_bass_kernel_spmd

DT = mybir.dt.float32
F32R = mybir.dt.float32r
AF = mybir.ActivationFunctionType
OP = mybir.AluOpType
BF16 = mybir.dt.bfloat16

N_CORES = 8
D = 1024
H = 16
HD = 64
FF = 4096
B = 2
S = 2048            # sequence length per batch
T = B * S           # 4096 total tokens
TOK = T // N_CORES  # 512 tokens per core in the token-sharded phase
LN_EPS = 1e-5
NEG = -30000.0

KT = D // 128        # 8 k-tiles over the model dim
DO = D // 128        # 8 output tiles over the model dim
NF = FF // 128       # 32 tiles over the ff dim
NQ = T // 512        # 8 column chunks of 512 tokens for QKV
SKT = S // 128       # 16 key tiles per batch sequence
SQT = S // 512       # 4 query tiles of 512 per batch sequence

LAST_RESULTS = None
_CACHE = {}


def _build_nc(debug=False):
    nc = bacc.Bacc("TRN2", target_bir_lowering=False, debug=False,
                   num_devices=N_CORES)

    # ---- DRAM I/O ----
    xT_d = nc.dram_tensor("xT", [KT, NQ, 128, 512], BF16, kind="ExternalInput")
    wqkv_d = nc.dram_tensor("wqkv", [128, KT, 3 * 128], BF16, kind="ExternalInput")
    bqkv_d = nc.dram_tensor("bqkv", [128, 3], DT, kind="ExternalInput")
    wo_d = nc.dram_tensor("wo", [128, KT, D], F32R, kind="ExternalInput")
    xres_d = nc.dram_tensor("xres", [128, DO, TOK], DT, kind="ExternalInput")
    w1_d = nc.dram_tensor("w1", [NF, 128, KT, 128], BF16, kind="ExternalInput")
    b1c_d = nc.dram_tensor("b1c", [128, NF], DT, kind="ExternalInput")
    w2_d = nc.dram_tensor("w2", [DO, 128, NF, 128], BF16, kind="ExternalInput")
    b2c_d = nc.dram_tensor("b2c", [128, DO], DT, kind="ExternalInput")
    gb1_d = nc.dram_tensor("gb1", [2, D], F32R, kind="ExternalInput")
    gb2_d = nc.dram_tensor("gb2", [2, D], F32R, kind="ExternalInput")
    masks_d = nc.dram_tensor("masks", [128, 128], DT, kind="ExternalInput")
    ident_d = nc.dram_tensor("ident64", [128, 64], F32R, kind="ExternalInput")
    ones128_d = nc.dram_tensor("ones128", [128, 1], F32R, kind="ExternalInput")
    onesv_d = nc.dram_tensor("onesv", [128, SKT], F32R, kind="ExternalInput")
    onesrow_d = nc.dram_tensor("onesrow", [1, TOK], F32R, kind="ExternalInput")
    y_d = nc.dram_tensor("y", [D, TOK], DT, kind="ExternalOutput")
    if debug:
        dbg_q = nc.dram_tensor("dbg_q", [128, T], DT, kind="ExternalOutput")
        dbg_k = nc.dram_tensor("dbg_k", [128, T], DT, kind="ExternalOutput")
        dbg_v = nc.dram_tensor("dbg_v", [128, T], DT, kind="ExternalOutput")
        dbg_ctx = nc.dram_tensor("dbg_ctx", [128, T], DT, kind="ExternalOutput")
        dbg_ctxf = nc.dram_tensor("dbg_ctxf", [D, TOK], DT, kind="ExternalOutput")
        dbg_h = nc.dram_tensor("dbg_h", [D, TOK], DT, kind="ExternalOutput")
        dbg_h1 = nc.dram_tensor("dbg_h1", [D, TOK], DT, kind="ExternalOutput")

    with tile.TileContext(nc) as tc:
        with (
            tc.tile_pool(name="const", bufs=1) as pconst,
            tc.tile_pool(name="dram", bufs=1, space="DRAM") as pdram,
        ):
            # ---- tiny persistent constants ----
            bqkv_sb = pconst.tile([128, 3], DT, tag="bqkv")
            nc.sync.dma_start(bqkv_sb[:], bqkv_d[:])
            ident_sb = pconst.tile([128, 64], F32R, tag="ident")
            nc.sync.dma_start(ident_sb[:], ident_d[:])
            ones128_sb = pconst.tile([128, 1], F32R, tag="ones128")
            nc.sync.dma_start(ones128_sb[:], ones128_d[:])
            gb1_sb = pconst.tile([2, D], F32R, tag="gb1")
            nc.sync.dma_start(gb1_sb[:], gb1_d[:])
            gb2_sb = pconst.tile([2, D], F32R, tag="gb2")
            nc.sync.dma_start(gb2_sb[:], gb2_d[:])
            b1c_sb = pconst.tile([128, NF], DT, tag="b1c")
            nc.sync.dma_start(b1c_sb[:], b1c_d[:])
            b2c_sb = pconst.tile([128, DO], DT, tag="b2c")
            nc.sync.dma_start(b2c_sb[:], b2c_d[:])

            # A2A DRAM bounce buffers (one per batch)
            HT = TOK // 2
            cc_in_a = pdram.tile([N_CORES, 128, HT], F32R, tag="cc_in_a")
            cc_out_a = pdram.tile([N_CORES, 128, HT], F32R, tag="cc_out_a")
            cc_in_b = pdram.tile([N_CORES, 128, HT], F32R, tag="cc_in_b")
            cc_out_b = pdram.tile([N_CORES, 128, HT], F32R, tag="cc_out_b")

            # ============ Phases 1-4: QKV, attention, A2A ============
            with (
                tc.tile_pool(name="attn", bufs=1) as pattn,
                tc.tile_pool(name="workA", bufs=3) as pwork,
                tc.tile_pool(name="workV", bufs=2) as pworkV,
            ):
                wqkv_sb = pattn.tile([128, KT, 3 * 128], BF16, tag="wqkv")
                nc.sync.dma_start(wqkv_sb[:], wqkv_d[:])
                masks_sb = pattn.tile([128, 128], DT, tag="masks")
                nc.sync.dma_start(masks_sb[:], masks_d[:])
                qT = pattn.tile([128, T], F32R, tag="qT")
                kT = pattn.tile([128, T], F32R, tag="kT")
                vT = pattn.tile([128, T], F32R, tag="vT")
                ctxT = pattn.tile([128, T], F32R, tag="ctxT")

                # -- Phase 1: QKV projections, V transposes interleaved --
                Vbs = {}
                for b in range(B):
                    for hl in range(2):
                        Vb = pworkV.tile([128, SKT, 65], F32R,
                                         tag=f"Vb{b}{hl}", name=f"Vb{b}{hl}")
                        nc.sync.dma_start(
                            Vb[:, :, 64:65].rearrange("p kt o -> p (kt o)"),
                            onesv_d[:])
                        Vbs[(b, hl)] = Vb

                def emit_vtr(m, psQ):
                    # V transposes for token chunk m (4 key tiles, 2 heads)
                    b = m // 4
                    for hl in range(2):
                        hoff = 64 * hl
                        for kl in range(4):
                            kt = (m % 4) * 4 + kl
                            gkt = 512 * m + 128 * kl
                            pv = psQ.tile([128, 64], F32R, tag="vtr",
                                          name="vtr")
                            nc.tensor.transpose(
                                pv[:],
                                vT[hoff:hoff + 64, gkt:gkt + 128],
                                ident_sb[hoff:hoff + 64, :])
                            nc.vector.tensor_copy(
                                Vbs[(b, hl)][:, kt, 0:64], pv[:])

                with tc.tile_pool(name="psQ", bufs=2, space="PSUM") as psQ:
                    for nq in range(NQ):
                        toff = 512 * nq
                        psq = psQ.tile([128, 512], DT, tag="ps_q")
                        psk = psQ.tile([128, 512], DT, tag="ps_k")
                        psv = psQ.tile([128, 512], DT, tag="ps_v")
                        pps = [psq, psk, psv]
                        for k in range(KT):
                            xk = pwork.tile([128, 512], BF16, tag="xk", bufs=6)
                            nc.sync.dma_start(xk[:], xT_d[k, nq])
                            for p in range(3):
                                nc.tensor.matmul(
                                    pps[p][:],
                                    wqkv_sb[:, k, 128 * p:128 * (p + 1)],
                                    xk[:],
                                    start=(k == 0), stop=(k == KT - 1))
                        for p, dest in ((0, qT), (1, kT), (2, vT)):
                            nc.scalar.activation(
                                dest[:, toff:toff + 512], pps[p][:],
                                AF.Identity, bias=bqkv_sb[:, p:p + 1])
                        if nq >= 1:
                            emit_vtr(nq - 1, psQ)
                    emit_vtr(NQ - 1, psQ)

                # -- Phases 2+3: per (batch) attention, heads fused --
                with (
                    tc.tile_pool(name="psS", bufs=3, space="PSUM") as psS,
                    tc.tile_pool(name="psC", bufs=1, space="PSUM") as psC,
                ):
                    for b in range(B):
                        boff = S * b
                        for qt in range(SQT):  # noqa: indent kept
                            qoff = boff + 512 * qt
                            nkt = 4 * (qt + 1)
                            ctxp = [psC.tile([65, 512], DT, tag=f"ctx{hl}",
                                             name=f"ctx{hl}")
                                    for hl in range(2)]
                            for kt in range(nkt):
                                j = kt - 4 * qt
                                c0 = max(0, 128 * j)   # first valid q col
                                scs = [psS.tile([128, 512], DT,
                                                tag=f"sc{hl}",
                                                name=f"sc{hl}")
                                       for hl in range(2)]
                                with tc.tile_critical():
                                    for hl in range(2):
                                        hoff = 64 * hl
                                        nc.tensor.matmul(
                                            scs[hl][:, c0:512],
                                            kT[hoff:hoff + 64,
                                               boff + 128 * kt:
                                               boff + 128 * (kt + 1)],
                                            qT[hoff:hoff + 64,
                                               qoff + c0:qoff + 512],
                                            start=True, stop=True)
                                exs = []
                                for hl in range(2):
                                    if j >= 0:
                                        nc.vector.tensor_add(
                                            scs[hl][:, c0:c0 + 128],
                                            scs[hl][:, c0:c0 + 128],
                                            masks_sb[:])
                                    ex = pwork.tile([128, 512], F32R,
                                                    tag=f"ex{hl}",
                                                    name=f"ex{hl}")
                                    nc.scalar.activation(
                                        ex[:, c0:512], scs[hl][:, c0:512],
                                        AF.Exp, scale=0.125)
                                    exs.append(ex)
                                for hl in range(2):
                                    nc.tensor.matmul(
                                        ctxp[hl][:, c0:512],
                                        Vbs[(b, hl)][:, kt, :],
                                        exs[hl][:, c0:512],
                                        start=(kt == 0),
                                        stop=(kt == nkt - 1))
                            for hl in range(2):
                                hoff = 64 * hl
                                srow = pwork.tile([1, 512], DT, tag="srow")
                                nc.vector.tensor_copy(srow[:],
                                                      ctxp[hl][64:65, :])
                                rec = pwork.tile([1, 512], DT, tag="rec")
                                nc.vector.reciprocal_approx_fast(
                                    rec[:], srow[:])
                                bc = pwork.tile([64, 512], DT, tag="bc")
                                nc.gpsimd.partition_broadcast(bc[:], rec[:])
                                nc.vector.tensor_mul(
                                    ctxT[hoff:hoff + 64, qoff:qoff + 512],
                                    ctxp[hl][0:64, :], bc[:])
                        # per-batch AllToAll: shard s = this batch's
                        # 256-token block s
                        cci = cc_in_a if b == 0 else cc_in_b
                        cco = cc_out_a if b == 0 else cc_out_b
                        nc.sync.dma_start(
                            cci[:].rearrange("s p t -> p s t"),
                            ctxT[:, boff:boff + S].rearrange(
                                "p (s t) -> p s t", s=N_CORES))
                        nc.gpsimd.collective_compute(
                            "AllToAll", OP.bypass,
                            ins=[cci[:].opt()],
                            outs=[cco[:].opt()],
                            replica_groups=[list(range(N_CORES))],
                        )

                if debug:
                    nc.sync.dma_start(dbg_q[:], qT[:].bitcast(DT))
                    nc.sync.dma_start(dbg_k[:], kT[:].bitcast(DT))
                    nc.sync.dma_start(dbg_v[:], vT[:].bitcast(DT))
                    nc.sync.dma_start(dbg_ctx[:], ctxT[:].bitcast(DT))

            _pmid_cm = tc.tile_pool(name="mid", bufs=1)
            pmid = _pmid_cm.__enter__()
            _pwork_cm = tc.tile_pool(name="workL", bufs=3)
            pwork = _pwork_cm.__enter__()
            h_sb = pmid.tile([128, DO, TOK], F32R, tag="h_sb")
            h1_sb = pmid.tile([128, DO, TOK], F32R, tag="h1_sb")

            def layernorm(gb_sb, out_writer, psT, psTc, h0=0, hw=TOK):
                """LN over h_sb cols [h0, h0+hw); out_writer(do, c1, c2)."""
                stat0 = psT.tile([1, hw], DT, tag="stat0", name="stat0")
                stat1 = psT.tile([1, hw], DT, tag="stat1", name="stat1")
                for k in range(DO):
                    h2 = pwork.tile([128, hw], F32R, tag="h2", name="h2")
                    nc.scalar.activation(h2[:],
                                         h_sb[:, k, h0:h0 + hw].bitcast(DT),
                                         AF.Square)
                    nc.tensor.matmul(stat0[:], ones128_sb[:],
                                     h_sb[:, k, h0:h0 + hw],
                                     start=(k == 0), stop=(k == DO - 1))
                    nc.tensor.matmul(stat1[:], ones128_sb[:], h2[:],
                                     start=(k == 0), stop=(k == DO - 1))
                rowsA = pwork.tile([1, hw], F32R, tag="rowsA", name="rowsA")
                rowsB2 = pwork.tile([2, hw], F32R, tag="rowsB2",
                                    name="rowsB2")  # B, ones
                nc.sync.dma_start(rowsB2[1:2, :], onesrow_d[:, 0:hw])
                mu = pwork.tile([1, hw], DT, tag="mu", name="mu")
                nc.vector.tensor_scalar_mul(mu[:], stat0[:], 1.0 / D)
                ex2 = pwork.tile([1, hw], DT, tag="ex2", name="ex2")
                nc.vector.tensor_scalar_mul(ex2[:], stat1[:], 1.0 / D)
                var = pwork.tile([1, hw], DT, tag="var", name="var")
                nc.vector.tensor_mul(var[:], mu[:], mu[:])
                nc.vector.tensor_sub(var[:], ex2[:], var[:])
                nc.vector.tensor_scalar_add(var[:], var[:], LN_EPS)
                sd = pwork.tile([1, hw], DT, tag="sd", name="sd")
                nc.scalar.activation(sd[:], var[:], AF.Sqrt)
                rs = pwork.tile([1, hw], DT, tag="rs", name="rs")
                nc.vector.reciprocal_approx_fast(rs[:], sd[:])
                nc.vector.tensor_copy(rowsA[:], rs[:])
                negmu = pwork.tile([1, hw], DT, tag="negmu", name="negmu")
                nc.vector.tensor_scalar_mul(negmu[:], mu[:], -1.0)
                nc.vector.tensor_mul(rowsB2[0:1, :], negmu[:], rs[:])
                for do in range(DO):
                    c1 = psTc.tile([128, hw], DT, tag="c1", name="c1")
                    nc.tensor.matmul(
                        c1[:], gb_sb[0:1, 128 * do:128 * (do + 1)],
                        rowsA[:], start=True, stop=True)
                    c2 = psTc.tile([128, hw], DT, tag="c2", name="c2")
                    nc.tensor.matmul(
                        c2[:], gb_sb[0:2, 128 * do:128 * (do + 1)],
                        rowsB2[:], start=True, stop=True)
                    out_writer(do, c1, c2)

            # ============ Phase 5: out-proj + residual + LN1 ============
            with (
                tc.tile_pool(name="p5", bufs=1) as p5,
                tc.tile_pool(name="psA", bufs=2, space="PSUM") as psA,
                tc.tile_pool(name="psT", bufs=1, space="PSUM") as psT,
                tc.tile_pool(name="psTc", bufs=2, space="PSUM") as psTc,
            ):
                wo_sb = p5.tile([128, KT, D], F32R, tag="wo")
                nc.sync.dma_start(wo_sb[:], wo_d[:])
                xres_sb = p5.tile([128, DO, TOK], DT, tag="xres")
                nc.sync.dma_start(xres_sb[:], xres_d[:])
                ctxf = p5.tile([128, KT, TOK], F32R, tag="ctxf")
                nc.sync.dma_start(
                    ctxf[:, :, 0:HT],
                    cc_out_a[:].rearrange("s p t -> p s t"))
                nc.sync.dma_start(
                    ctxf[:, :, HT:TOK],
                    cc_out_b[:].rearrange("s p t -> p s t"))
                for do in range(DO):
                    acc = psA.tile([128, TOK], DT, tag="acc")
                    for half in range(2):
                        h0 = HT * half
                        for k in range(KT):
                            nc.tensor.matmul(
                                acc[:, h0:h0 + HT],
                                wo_sb[:, k, 128 * do:128 * (do + 1)],
                                ctxf[:, k, h0:h0 + HT],
                                start=(k == 0), stop=(k == KT - 1))
                    nc.vector.tensor_add(h_sb[:, do, :], xres_sb[:, do, :],
                                         acc[:])

                if debug:
                    nc.sync.dma_start(
                        dbg_ctxf[:].rearrange("(kt p) n -> p kt n", p=128),
                        ctxf[:].bitcast(DT))
                    nc.sync.dma_start(
                        dbg_h[:].rearrange("(kt p) n -> p kt n", p=128),
                        h_sb[:].bitcast(DT))

                def write_h1(do, c1, c2):
                    tmp = pwork.tile([128, TOK], DT, tag="lntmp")
                    nc.vector.tensor_mul(tmp[:], h_sb[:, do, :].bitcast(DT),
                                         c1[:])
                    nc.vector.tensor_add(h1_sb[:, do, :], tmp[:], c2[:])

                layernorm(gb1_sb, write_h1, psT, psTc)
                if debug:
                    nc.sync.dma_start(
                        dbg_h1[:].rearrange("(kt p) n -> p kt n", p=128),
                        h1_sb[:].bitcast(DT))

            # ============ Phases 6-7: MLP + LN2 ============
            with (
                tc.tile_pool(name="pfc", bufs=1) as pfc,
                tc.tile_pool(name="pw", bufs=2) as pw,
                tc.tile_pool(name="psA2", bufs=2, space="PSUM") as psA2,
                tc.tile_pool(name="psT2", bufs=1, space="PSUM") as psT2,
                tc.tile_pool(name="psT2c", bufs=2, space="PSUM") as psT2c,
            ):
                h1b = pfc.tile([128, DO, TOK], BF16, tag="h1b")
                for k in range(DO):
                    nc.vector.tensor_copy(h1b[:, k, :],
                                          h1_sb[:, k, :].bitcast(DT))
                a_sb = pfc.tile([128, NF, TOK], BF16, tag="a_sb")
                for ff in range(NF):
                    w1b = pw.tile([128, KT, 128], BF16, tag="w1b", bufs=3)
                    nc.sync.dma_start(w1b[:], w1_d[ff])
                    acc = psA2.tile([128, TOK], DT, tag="acc")
                    for k in range(KT):
                        nc.tensor.matmul(
                            acc[:], w1b[:, k, :], h1b[:, k, :],
                            start=(k == 0), stop=(k == KT - 1))
                    nc.scalar.activation(a_sb[:, ff, :], acc[:], AF.Gelu,
                                         bias=b1c_sb[:, ff:ff + 1])

                NFH = NF // 2
                for do in range(DO):
                    acc = psA2.tile([128, TOK], DT, tag="acc")
                    for half in range(2):
                        w2b = pw.tile([128, NFH, 128], BF16, tag="w2b")
                        nc.sync.dma_start(
                            w2b[:], w2_d[do, :, NFH * half:NFH * (half + 1), :])
                        for k in range(NFH):
                            kk = NFH * half + k
                            nc.tensor.matmul(
                                acc[:], w2b[:, k, :], a_sb[:, kk, :],
                                start=(kk == 0),
                                stop=(kk == NF - 1))
                    tmpf = pwork.tile([128, TOK], DT, tag="f2tmp")
                    nc.scalar.activation(tmpf[:], acc[:], AF.Identity,
                                         bias=b2c_sb[:, do:do + 1])
                    nc.vector.tensor_add(h_sb[:, do, :], tmpf[:],
                                         h1_sb[:, do, :].bitcast(DT))

                def write_out(do, c1, c2):
                    tmp = pwork.tile([128, TOK], DT, tag="lntmp")
                    nc.vector.tensor_mul(tmp[:], h_sb[:, do, :].bitcast(DT),
                                         c1[:])
                    out_t = pwork.tile([128, TOK], DT, tag="out_t")
                    nc.vector.tensor_add(out_t[:], tmp[:], c2[:])
                    nc.sync.dma_start(
                        y_d[128 * do:128 * (do + 1), :], out_t[:])

                layernorm(gb2_sb, write_out, psT2, psT2c)

            _pwork_cm.__exit__(None, None, None)
            _pmid_cm.__exit__(None, None, None)

    nc.compile()
    return nc


def kernel(**inputs):
    global LAST_RESULTS
    import os
    debug = bool(os.environ.get("KERNEL_DEBUG"))
    key = ("nc", debug)
    if key not in _CACHE:
        _CACHE[key] = _build_nc(debug)
    nc = _CACHE[key]

    f32 = np.float32
    x = np.asarray(inputs["x"], f32)
    Wq, bq = np.asarray(inputs["Wq"], f32), np.asarray(inputs["bq"], f32)
    Wk, bk = np.asarray(inputs["Wk"], f32), np.asarray(inputs["bk"], f32)
    Wv, bv = np.asarray(inputs["Wv"], f32), np.asarray(inputs["bv"], f32)
    Wo, bo = np.asarray(inputs["Wo"], f32), np.asarray(inputs["bo"], f32)
    ln1_g, ln1_b = np.asarray(inputs["ln1_g"], f32), np.asarray(inputs["ln1_b"], f32)
    W1, b1 = np.asarray(inputs["W1"], f32), np.asarray(inputs["b1"], f32)
    W2, b2 = np.asarray(inputs["W2"], f32), np.asarray(inputs["b2"], f32)
    ln2_g, ln2_b = np.asarray(inputs["ln2_g"], f32), np.asarray(inputs["ln2_b"], f32)

    xT = np.ascontiguousarray(x.reshape(T, D).T)          # [D, T]
    import ml_dtypes
    bf16 = ml_dtypes.bfloat16
    xTt = np.ascontiguousarray(
        xT.reshape(KT, 128, NQ, 512).transpose(0, 2, 1, 3)).astype(bf16)

    # single [k, q] triangular mask for diagonal 128-blocks
    kk = np.arange(128)[:, None]
    qq = np.arange(128)[None, :]
    masks = np.where(qq >= kk, 0.0, NEG).astype(f32)

    shared = {
        "xT": xTt,
        "wo": np.ascontiguousarray(Wo.reshape(KT, 128, D).transpose(1, 0, 2)),
        "w1": np.ascontiguousarray(
            W1.reshape(KT, 128, NF, 128).transpose(2, 1, 0, 3)).astype(bf16),
        "b1c": np.ascontiguousarray(b1.reshape(NF, 128).T),
        "w2": np.ascontiguousarray(
            W2.reshape(NF, 128, DO, 128).transpose(2, 1, 0, 3)).astype(bf16),
        "b2c": np.ascontiguousarray(b2.reshape(DO, 128).T),
        "gb1": np.ascontiguousarray(np.stack([ln1_g, ln1_b])),
        "gb2": np.ascontiguousarray(np.stack([ln2_g, ln2_b])),
        "masks": masks,
        "ident64": np.concatenate([np.eye(64, dtype=f32)] * 2, axis=0),
        "ones128": np.ones((128, 1), f32),
        "onesv": np.ones((128, SKT), f32),
        "onesrow": np.ones((1, TOK), f32),
    }

    in_maps = []
    for c in range(N_CORES):
        cs = slice(128 * c, 128 * (c + 1))       # this core's 2 heads' dims
        wqkv = np.concatenate([Wq[:, cs], Wk[:, cs], Wv[:, cs]], axis=1)
        wqkv = np.ascontiguousarray(
            wqkv.reshape(KT, 128, 3 * 128).transpose(1, 0, 2)).astype(bf16)
        bqkv = np.stack([bq[cs], bk[cs], bv[cs]], axis=1)   # [128, 3]
        xres = np.concatenate(
            [xT[:, 256 * c:256 * (c + 1)],
             xT[:, 2048 + 256 * c:2048 + 256 * (c + 1)]],
            axis=1) + bo[:, None]
        xres = np.ascontiguousarray(
            xres.reshape(DO, 128, TOK).transpose(1, 0, 2))
        in_maps.append({
            **shared,
            "wqkv": wqkv,
            "bqkv": np.ascontiguousarray(bqkv),
            "xres": xres,
        })

    res = run_bass_kernel_spmd(nc, in_maps, core_ids=list(range(N_CORES)))
    LAST_RESULTS = res

    outT = np.empty((D, T), np.float32)
    for c in range(N_CORES):
        yc = res.results[c]["y"]
        outT[:, 256 * c:256 * (c + 1)] = yc[:, 0:256]
        outT[:, 2048 + 256 * c:2048 + 256 * (c + 1)] = yc[:, 256:512]
    return np.ascontiguousarray(outT.T).reshape(B, S, D)


# revision 21
# speedup vs baseline: 1.2925x; 1.2925x over previous
"""Fused transformer decoder block on 8 Trainium2 NeuronCores.

Sharding: Ulysses-style. Each core computes QKV projections for its 2 heads
(x2 batches) over all tokens, runs causal attention for those heads, then an
AllToAll redistributes context from head-sharded to token-sharded; each core
finishes out-proj + LN1 + MLP + LN2 for its 512-token shard.

All activations live in transposed layout (model dim on partitions, tokens on
the free axis) so every matmul contraction runs over the partition dim with
no activation transposes. Matmuls use float32r (TF32-like, full PE rate).
"""

import sys

sys.path.insert(0, "/opt/trn_rl_repo")

import numpy as np

import concourse.bass as bass
import concourse.mybir as mybir
import concourse.tile as tile
from concourse import bacc
from concourse.bass_utils import run_bass_kernel_spmd

DT = mybir.dt.float32
F32R = mybir.dt.float32r
AF = mybir.ActivationFunctionType
OP = mybir.AluOpType
BF16 = mybir.dt.bfloat16

N_CORES = 8
D = 1024
H = 16
HD = 64
FF = 4096
B = 2
S = 2048            # sequence length per batch
T = B * S           # 4096 total tokens
TOK = T // N_CORES  # 512 tokens per core in the token-sharded phase
LN_EPS = 1e-5
NEG = -30000.0

KT = D // 128        # 8 k-tiles over the model dim
DO = D // 128        # 8 output tiles over the model dim
NF = FF // 128       # 32 tiles over the ff dim
NQ = T // 512        # 8 column chunks of 512 tokens for QKV
SKT = S // 128       # 16 key tiles per batch sequence
SQT = S // 512       # 4 query tiles of 512 per batch sequence

LAST_RESULTS = None
_CACHE = {}


def _build_nc(debug=False):
    nc = bacc.Bacc("TRN2", target_bir_lowering=False, debug=False,
                   num_devices=N_CORES)

    # ---- DRAM I/O ----
    xT_d = nc.dram_tensor("xT", [KT, NQ, 128, 512], BF16, kind="ExternalInput")
    wqkv_d = nc.dram_tensor("wqkv", [128, KT, 3 * 128], BF16, kind="ExternalInput")
    bqkv_d = nc.dram_tensor("bqkv", [128, 3], DT, kind="ExternalInput")
    wo_d = nc.dram_tensor("wo", [128, KT, D], F32R, kind="ExternalInput")
    xres_d = nc.dram_tensor("xres", [128, DO, TOK], DT, kind="ExternalInput")
    w1_d = nc.dram_tensor("w1", [NF, 128, KT, 128], BF16, kind="ExternalInput")
    b1c_d = nc.dram_tensor("b1c", [128, NF], DT, kind="ExternalInput")
    w2_d = nc.dram_tensor("w2", [DO, 128, NF, 128], BF16, kind="ExternalInput")
    b2c_d = nc.dram_tensor("b2c", [128, DO], DT, kind="ExternalInput")
    gb1_d = nc.dram_tensor("gb1", [2, D], F32R, kind="ExternalInput")
    gb2_d = nc.dram_tensor("gb2", [2, D], F32R, kind="ExternalInput")
    masks_d = nc.dram_tensor("masks", [128, 128], DT, kind="ExternalInput")
    ident_d = nc.dram_tensor("ident64", [128, 64], F32R, kind="ExternalInput")
    ones128_d = nc.dram_tensor("ones128", [128, 1], F32R, kind="ExternalInput")
    onesv_d = nc.dram_tensor("onesv", [128, SKT], F32R, kind="ExternalInput")
    onesrow_d = nc.dram_tensor("onesrow", [1, TOK], F32R, kind="ExternalInput")
    y_d = nc.dram_tensor("y", [D, TOK], DT, kind="ExternalOutput")
    if debug:
        dbg_q = nc.dram_tensor("dbg_q", [128, T], DT, kind="ExternalOutput")
        dbg_k = nc.dram_tensor("dbg_k", [128, T], DT, kind="ExternalOutput")
        dbg_v = nc.dram_tensor("dbg_v", [128, T], DT, kind="ExternalOutput")
        dbg_ctx = nc.dram_tensor("dbg_ctx", [128, T], DT, kind="ExternalOutput")
        dbg_ctxf = nc.dram_tensor("dbg_ctxf", [D, TOK], DT, kind="ExternalOutput")
        dbg_h = nc.dram_tensor("dbg_h", [D, TOK], DT, kind="ExternalOutput")
        dbg_h1 = nc.dram_tensor("dbg_h1", [D, TOK], DT, kind="ExternalOutput")

    with tile.TileContext(nc) as tc:
        with (
            tc.tile_pool(name="const", bufs=1) as pconst,
            tc.tile_pool(name="dram", bufs=1, space="DRAM") as pdram,
        ):
            # ---- tiny persistent constants ----
            bqkv_sb = pconst.tile([128, 3], DT, tag="bqkv")
            nc.sync.dma_start(bqkv_sb[:], bqkv_d[:])
            ident_sb = pconst.tile([128, 64], F32R, tag="ident")
            nc.sync.dma_start(ident_sb[:], ident_d[:])
            ones128_sb = pconst.tile([128, 1], F32R, tag="ones128")
            nc.sync.dma_start(ones128_sb[:], ones128_d[:])
            gb1_sb = pconst.tile([2, D], F32R, tag="gb1")
            nc.sync.dma_start(gb1_sb[:], gb1_d[:])
            gb2_sb = pconst.tile([2, D], F32R, tag="gb2")
            nc.sync.dma_start(gb2_sb[:], gb2_d[:])
            b1c_sb = pconst.tile([128, NF], DT, tag="b1c")
            nc.sync.dma_start(b1c_sb[:], b1c_d[:])
            b2c_sb = pconst.tile([128, DO], DT, tag="b2c")
            nc.sync.dma_start(b2c_sb[:], b2c_d[:])

            # A2A DRAM bounce buffers (one per batch)
            HT = TOK // 2
            cc_in_a = pdram.tile([N_CORES, 128, HT], F32R, tag="cc_in_a")
            cc_out_a = pdram.tile([N_CORES, 128, HT], F32R, tag="cc_out_a")
            cc_in_b = pdram.tile([N_CORES, 128, HT], F32R, tag="cc_in_b")
            cc_out_b = pdram.tile([N_CORES, 128, HT], F32R, tag="cc_out_b")

            # ============ Phases 1-4: QKV, attention, A2A ============
            with (
                tc.tile_pool(name="attn", bufs=1) as pattn,
                tc.tile_pool(name="workA", bufs=3) as pwork,
                tc.tile_pool(name="workV", bufs=2) as pworkV,
            ):
                wqkv_sb = pattn.tile([128, KT, 3 * 128], BF16, tag="wqkv")
                nc.sync.dma_start(wqkv_sb[:], wqkv_d[:])
                masks_sb = pattn.tile([128, 128], DT, tag="masks")
                nc.sync.dma_start(masks_sb[:], masks_d[:])
                qTb = pattn.tile([128, T], BF16, tag="qTb")
                kTz = pattn.tile([128, 2, T], BF16, tag="kTz")
                vT = pattn.tile([128, T], F32R, tag="vT")
                ctxT = pattn.tile([128, T], F32R, tag="ctxT")
                # zero the off-head halves of kTz once
                nc.vector.memset(kTz[64:128, 0, :], 0.0)
                nc.vector.memset(kTz[0:64, 1, :], 0.0)

                # -- Phase 1: QKV projections, V transposes interleaved --
                Vbs = {}
                for b in range(B):
                    for hl in range(2):
                        Vb = pworkV.tile([128, SKT, 65], F32R,
                                         tag=f"Vb{b}{hl}", name=f"Vb{b}{hl}")
                        nc.sync.dma_start(
                            Vb[:, :, 64:65].rearrange("p kt o -> p (kt o)"),
                            onesv_d[:])
                        Vbs[(b, hl)] = Vb

                def emit_vtr(m, psQ):
                    # V transposes for token chunk m (4 key tiles, 2 heads)
                    b = m // 4
                    for hl in range(2):
                        hoff = 64 * hl
                        for kl in range(4):
                            kt = (m % 4) * 4 + kl
                            gkt = 512 * m + 128 * kl
                            pv = psQ.tile([128, 64], F32R, tag="vtr",
                                          name="vtr")
                            nc.tensor.transpose(
                                pv[:],
                                vT[hoff:hoff + 64, gkt:gkt + 128],
                                ident_sb[hoff:hoff + 64, :])
                            nc.vector.tensor_copy(
                                Vbs[(b, hl)][:, kt, 0:64], pv[:])

                with tc.tile_pool(name="psQ", bufs=2, space="PSUM") as psQ:
                    for nq in range(NQ):
                        toff = 512 * nq
                        psq = psQ.tile([128, 512], DT, tag="ps_q")
                        psk = psQ.tile([128, 512], DT, tag="ps_k")
                        psv = psQ.tile([128, 512], DT, tag="ps_v")
                        pps = [psq, psk, psv]
                        for k in range(KT):
                            xk = pwork.tile([128, 512], BF16, tag="xk", bufs=6)
                            nc.sync.dma_start(xk[:], xT_d[k, nq])
                            for p in range(3):
                                nc.tensor.matmul(
                                    pps[p][:],
                                    wqkv_sb[:, k, 128 * p:128 * (p + 1)],
                                    xk[:],
                                    start=(k == 0), stop=(k == KT - 1))
                        nc.scalar.activation(
                            qTb[:, toff:toff + 512], pps[0][:],
                            AF.Identity, bias=bqkv_sb[:, 0:1])
                        nc.scalar.activation(
                            kTz[0:64, 0, toff:toff + 512], pps[1][0:64, :],
                            AF.Identity, bias=bqkv_sb[0:64, 1:2])
                        nc.scalar.activation(
                            kTz[64:128, 1, toff:toff + 512],
                            pps[1][64:128, :],
                            AF.Identity, bias=bqkv_sb[64:128, 1:2])
                        nc.scalar.activation(
                            vT[:, toff:toff + 512], pps[2][:],
                            AF.Identity, bias=bqkv_sb[:, 2:3])
                        if nq >= 1:
                            emit_vtr(nq - 1, psQ)
                    emit_vtr(NQ - 1, psQ)

                # -- Phases 2+3: per (batch) attention, heads fused --
                with (
                    tc.tile_pool(name="psS", bufs=3, space="PSUM") as psS,
                    tc.tile_pool(name="psC", bufs=1, space="PSUM") as psC,
                ):
                    for b in range(B):
                        boff = S * b
                        for qt in range(SQT):  # noqa: indent kept
                            qoff = boff + 512 * qt
                            nkt = 4 * (qt + 1)
                            ctxp = [psC.tile([65, 512], DT, tag=f"ctx{hl}",
                                             name=f"ctx{hl}")
                                    for hl in range(2)]
                            for kt in range(nkt):
                                j = kt - 4 * qt
                                c0 = max(0, 128 * j)   # first valid q col
                                scs = [psS.tile([128, 512], DT,
                                                tag=f"sc{hl}",
                                                name=f"sc{hl}")
                                       for hl in range(2)]
                                for hl in range(2):
                                    nc.tensor.matmul(
                                        scs[hl][:, c0:512],
                                        kTz[:, hl,
                                            boff + 128 * kt:
                                            boff + 128 * (kt + 1)],
                                        qTb[:, qoff + c0:qoff + 512],
                                        start=True, stop=True)
                                exs = []
                                for hl in range(2):
                                    if j >= 0:
                                        nc.vector.tensor_add(
                                            scs[hl][:, c0:c0 + 128],
                                            scs[hl][:, c0:c0 + 128],
                                            masks_sb[:])
                                    ex = pwork.tile([128, 512], F32R,
                                                    tag=f"ex{hl}",
                                                    name=f"ex{hl}")
                                    nc.scalar.activation(
                                        ex[:, c0:512], scs[hl][:, c0:512],
                                        AF.Exp, scale=0.125)
                                    exs.append(ex)
                                for hl in range(2):
                                    nc.tensor.matmul(
                                        ctxp[hl][:, c0:512],
                                        Vbs[(b, hl)][:, kt, :],
                                        exs[hl][:, c0:512],
                                        start=(kt == 0),
                                        stop=(kt == nkt - 1))
                            for hl in range(2):
                                hoff = 64 * hl
                                srow = pwork.tile([1, 512], DT, tag="srow")
                                nc.vector.tensor_copy(srow[:],
                                                      ctxp[hl][64:65, :])
                                rec = pwork.tile([1, 512], DT, tag="rec")
                                nc.vector.reciprocal_approx_fast(
                                    rec[:], srow[:])
                                bc = pwork.tile([64, 512], DT, tag="bc")
                                nc.gpsimd.partition_broadcast(bc[:], rec[:])
                                nc.vector.tensor_mul(
                                    ctxT[hoff:hoff + 64, qoff:qoff + 512],
                                    ctxp[hl][0:64, :], bc[:])
                        # per-batch AllToAll: shard s = this batch's
                        # 256-token block s
                        cci = cc_in_a if b == 0 else cc_in_b
                        cco = cc_out_a if b == 0 else cc_out_b
                        nc.sync.dma_start(
                            cci[:].rearrange("s p t -> p s t"),
                            ctxT[:, boff:boff + S].rearrange(
                                "p (s t) -> p s t", s=N_CORES))
                        nc.gpsimd.collective_compute(
                            "AllToAll", OP.bypass,
                            ins=[cci[:].opt()],
                            outs=[cco[:].opt()],
                            replica_groups=[list(range(N_CORES))],
                        )

                if debug:
                    nc.sync.dma_start(dbg_v[:], vT[:].bitcast(DT))
                    nc.sync.dma_start(dbg_ctx[:], ctxT[:].bitcast(DT))

            _pmid_cm = tc.tile_pool(name="mid", bufs=1)
            pmid = _pmid_cm.__enter__()
            _pwork_cm = tc.tile_pool(name="workL", bufs=3)
            pwork = _pwork_cm.__enter__()
            h_sb = pmid.tile([128, DO, TOK], F32R, tag="h_sb")
            h1_sb = pmid.tile([128, DO, TOK], F32R, tag="h1_sb")

            def layernorm(gb_sb, out_writer, psT, psTc, h0=0, hw=TOK):
                """LN over h_sb cols [h0, h0+hw); out_writer(do, c1, c2)."""
                stat0 = psT.tile([1, hw], DT, tag="stat0", name="stat0")
                stat1 = psT.tile([1, hw], DT, tag="stat1", name="stat1")
                for k in range(DO):
                    h2 = pwork.tile([128, hw], F32R, tag="h2", name="h2")
                    nc.scalar.activation(h2[:],
                                         h_sb[:, k, h0:h0 + hw].bitcast(DT),
                                         AF.Square)
                    nc.tensor.matmul(stat0[:], ones128_sb[:],
                                     h_sb[:, k, h0:h0 + hw],
                                     start=(k == 0), stop=(k == DO - 1))
                    nc.tensor.matmul(stat1[:], ones128_sb[:], h2[:],
                                     start=(k == 0), stop=(k == DO - 1))
                rowsA = pwork.tile([1, hw], F32R, tag="rowsA", name="rowsA")
                rowsB2 = pwork.tile([2, hw], F32R, tag="rowsB2",
                                    name="rowsB2")  # B, ones
                nc.sync.dma_start(rowsB2[1:2, :], onesrow_d[:, 0:hw])
                mu = pwork.tile([1, hw], DT, tag="mu", name="mu")
                nc.vector.tensor_scalar_mul(mu[:], stat0[:], 1.0 / D)
                ex2 = pwork.tile([1, hw], DT, tag="ex2", name="ex2")
                nc.vector.tensor_scalar_mul(ex2[:], stat1[:], 1.0 / D)
                var = pwork.tile([1, hw], DT, tag="var", name="var")
                nc.vector.tensor_mul(var[:], mu[:], mu[:])
                nc.vector.tensor_sub(var[:], ex2[:], var[:])
                nc.vector.tensor_scalar_add(var[:], var[:], LN_EPS)
                sd = pwork.tile([1, hw], DT, tag="sd", name="sd")
                nc.scalar.activation(sd[:], var[:], AF.Sqrt)
                rs = pwork.tile([1, hw], DT, tag="rs", name="rs")
                nc.vector.reciprocal_approx_fast(rs[:], sd[:])
                nc.vector.tensor_copy(rowsA[:], rs[:])
                negmu = pwork.tile([1, hw], DT, tag="negmu", name="negmu")
                nc.vector.tensor_scalar_mul(negmu[:], mu[:], -1.0)
                nc.vector.tensor_mul(rowsB2[0:1, :], negmu[:], rs[:])
                for do in range(DO):
                    c1 = psTc.tile([128, hw], DT, tag="c1", name="c1")
                    nc.tensor.matmul(
                        c1[:], gb_sb[0:1, 128 * do:128 * (do + 1)],
                        rowsA[:], start=True, stop=True)
                    c2 = psTc.tile([128, hw], DT, tag="c2", name="c2")
                    nc.tensor.matmul(
                        c2[:], gb_sb[0:2, 128 * do:128 * (do + 1)],
                        rowsB2[:], start=True, stop=True)
                    out_writer(do, c1, c2)

            # ============ Phase 5: out-proj + residual + LN1 ============
            with (
                tc.tile_pool(name="p5", bufs=1) as p5,
                tc.tile_pool(name="psA", bufs=2, space="PSUM") as psA,
                tc.tile_pool(name="psT", bufs=1, space="PSUM") as psT,
                tc.tile_pool(name="psTc", bufs=2, space="PSUM") as psTc,
            ):
                wo_sb = p5.tile([128, KT, D], F32R, tag="wo")
                nc.sync.dma_start(wo_sb[:], wo_d[:])
                xres_sb = p5.tile([128, DO, TOK], DT, tag="xres")
                nc.sync.dma_start(xres_sb[:], xres_d[:])
                ctxf = p5.tile([128, KT, TOK], F32R, tag="ctxf")
                nc.sync.dma_start(
                    ctxf[:, :, 0:HT],
                    cc_out_a[:].rearrange("s p t -> p s t"))
                nc.sync.dma_start(
                    ctxf[:, :, HT:TOK],
                    cc_out_b[:].rearrange("s p t -> p s t"))
                for do in range(DO):
                    acc = psA.tile([128, TOK], DT, tag="acc")
                    for half in range(2):
                        h0 = HT * half
                        for k in range(KT):
                            nc.tensor.matmul(
                                acc[:, h0:h0 + HT],
                                wo_sb[:, k, 128 * do:128 * (do + 1)],
                                ctxf[:, k, h0:h0 + HT],
                                start=(k == 0), stop=(k == KT - 1))
                    nc.vector.tensor_add(h_sb[:, do, :], xres_sb[:, do, :],
                                         acc[:])

                if debug:
                    nc.sync.dma_start(
                        dbg_ctxf[:].rearrange("(kt p) n -> p kt n", p=128),
                        ctxf[:].bitcast(DT))
                    nc.sync.dma_start(
                        dbg_h[:].rearrange("(kt p) n -> p kt n", p=128),
                        h_sb[:].bitcast(DT))

                def write_h1(do, c1, c2):
                    tmp = pwork.tile([128, TOK], DT, tag="lntmp")
                    nc.vector.tensor_mul(tmp[:], h_sb[:, do, :].bitcast(DT),
                                         c1[:])
                    nc.vector.tensor_add(h1_sb[:, do, :], tmp[:], c2[:])

                layernorm(gb1_sb, write_h1, psT, psTc)
                if debug:
                    nc.sync.dma_start(
                        dbg_h1[:].rearrange("(kt p) n -> p kt n", p=128),
                        h1_sb[:].bitcast(DT))

            # ============ Phases 6-7: MLP + LN2 ============
            with (
                tc.tile_pool(name="pfc", bufs=1) as pfc,
                tc.tile_pool(name="pw", bufs=2) as pw,
                tc.tile_pool(name="psA2", bufs=2, space="PSUM") as psA2,
                tc.tile_pool(name="psT2", bufs=1, space="PSUM") as psT2,
                tc.tile_pool(name="psT2c", bufs=2, space="PSUM") as psT2c,
            ):
                h1b = pfc.tile([128, DO, TOK], BF16, tag="h1b")
                for k in range(DO):
                    nc.vector.tensor_copy(h1b[:, k, :],
                                          h1_sb[:, k, :].bitcast(DT))
                a_sb = pfc.tile([128, NF, TOK], BF16, tag="a_sb")
                for ff in range(NF):
                    w1b = pw.tile([128, KT, 128], BF16, tag="w1b", bufs=3)
                    nc.sync.dma_start(w1b[:], w1_d[ff])
                    acc = psA2.tile([128, TOK], DT, tag="acc")
                    for k in range(KT):
                        nc.tensor.matmul(
                            acc[:], w1b[:, k, :], h1b[:, k, :],
                            start=(k == 0), stop=(k == KT - 1))
                    nc.scalar.activation(a_sb[:, ff, :], acc[:], AF.Gelu,
                                         bias=b1c_sb[:, ff:ff + 1])

                NFH = NF // 2
                for do in range(DO):
                    acc = psA2.tile([128, TOK], DT, tag="acc")
                    for half in range(2):
                        w2b = pw.tile([128, NFH, 128], BF16, tag="w2b")
                        nc.sync.dma_start(
                            w2b[:], w2_d[do, :, NFH * half:NFH * (half + 1), :])
                        for k in range(NFH):
                            kk = NFH * half + k
                            nc.tensor.matmul(
                                acc[:], w2b[:, k, :], a_sb[:, kk, :],
                                start=(kk == 0),
                                stop=(kk == NF - 1))
                    tmpf = pwork.tile([128, TOK], DT, tag="f2tmp")
                    nc.scalar.activation(tmpf[:], acc[:], AF.Identity,
                                         bias=b2c_sb[:, do:do + 1])
                    nc.vector.tensor_add(h_sb[:, do, :], tmpf[:],
                                         h1_sb[:, do, :].bitcast(DT))

                def write_out(do, c1, c2):
                    tmp = pwork.tile([128, TOK], DT, tag="lntmp")
                    nc.vector.tensor_mul(tmp[:], h_sb[:, do, :].bitcast(DT),
                                         c1[:])
                    out_t = pwork.tile([128, TOK], DT, tag="out_t")
                    nc.vector.tensor_add(out_t[:], tmp[:], c2[:])
                    nc.sync.dma_start(
                        y_d[128 * do:128 * (do + 1), :], out_t[:])

                layernorm(gb2_sb, write_out, psT2, psT2c)

            _pwork_cm.__exit__(None, None, None)
            _pmid_cm.__exit__(None, None, None)

    nc.compile()
    return nc


def kernel(**inputs):
    global LAST_RESULTS
    import os
    debug = bool(os.environ.get("KERNEL_DEBUG"))
    key = ("nc", debug)
    if key not in _CACHE:
        _CACHE[key] = _build_nc(debug)
    nc = _CACHE[key]

    f32 = np.float32
    x = np.asarray(inputs["x"], f32)
    Wq, bq = np.asarray(inputs["Wq"], f32), np.asarray(inputs["bq"], f32)
    Wk, bk = np.asarray(inputs["Wk"], f32), np.asarray(inputs["bk"], f32)
    Wv, bv = np.asarray(inputs["Wv"], f32), np.asarray(inputs["bv"], f32)
    Wo, bo = np.asarray(inputs["Wo"], f32), np.asarray(inputs["bo"], f32)
    ln1_g, ln1_b = np.asarray(inputs["ln1_g"], f32), np.asarray(inputs["ln1_b"], f32)
    W1, b1 = np.asarray(inputs["W1"], f32), np.asarray(inputs["b1"], f32)
    W2, b2 = np.asarray(inputs["W2"], f32), np.asarray(inputs["b2"], f32)
    ln2_g, ln2_b = np.asarray(inputs["ln2_g"], f32), np.asarray(inputs["ln2_b"], f32)

    xT = np.ascontiguousarray(x.reshape(T, D).T)          # [D, T]
    import ml_dtypes
    bf16 = ml_dtypes.bfloat16
    xTt = np.ascontiguousarray(
        xT.reshape(KT, 128, NQ, 512).transpose(0, 2, 1, 3)).astype(bf16)

    # single [k, q] triangular mask for diagonal 128-blocks
    kk = np.arange(128)[:, None]
    qq = np.arange(128)[None, :]
    masks = np.where(qq >= kk, 0.0, NEG).astype(f32)

    shared = {
        "xT": xTt,
        "wo": np.ascontiguousarray(Wo.reshape(KT, 128, D).transpose(1, 0, 2)),
        "w1": np.ascontiguousarray(
            W1.reshape(KT, 128, NF, 128).transpose(2, 1, 0, 3)).astype(bf16),
        "b1c": np.ascontiguousarray(b1.reshape(NF, 128).T),
        "w2": np.ascontiguousarray(
            W2.reshape(NF, 128, DO, 128).transpose(2, 1, 0, 3)).astype(bf16),
        "b2c": np.ascontiguousarray(b2.reshape(DO, 128).T),
        "gb1": np.ascontiguousarray(np.stack([ln1_g, ln1_b])),
        "gb2": np.ascontiguousarray(np.stack([ln2_g, ln2_b])),
        "masks": masks,
        "ident64": np.concatenate([np.eye(64, dtype=f32)] * 2, axis=0),
        "ones128": np.ones((128, 1), f32),
        "onesv": np.ones((128, SKT), f32),
        "onesrow": np.ones((1, TOK), f32),
    }

    in_maps = []
    for c in range(N_CORES):
        cs = slice(128 * c, 128 * (c + 1))       # this core's 2 heads' dims
        wqkv = np.concatenate([Wq[:, cs], Wk[:, cs], Wv[:, cs]], axis=1)
        wqkv = np.ascontiguousarray(
            wqkv.reshape(KT, 128, 3 * 128).transpose(1, 0, 2)).astype(bf16)
        bqkv = np.stack([bq[cs], bk[cs], bv[cs]], axis=1)   # [128, 3]
        xres = np.concatenate(
            [xT[:, 256 * c:256 * (c + 1)],
             xT[:, 2048 + 256 * c:2048 + 256 * (c + 1)]],
            axis=1) + bo[:, None]
        xres = np.ascontiguousarray(
            xres.reshape(DO, 128, TOK).transpose(1, 0, 2))
        in_maps.append({
            **shared,
            "wqkv": wqkv,
            "bqkv": np.ascontiguousarray(bqkv),
            "xres": xres,
        })

    res = run_bass_kernel_spmd(nc, in_maps, core_ids=list(range(N_CORES)))
    LAST_RESULTS = res

    outT = np.empty((D, T), np.float32)
    for c in range(N_CORES):
        yc = res.results[c]["y"]
        outT[:, 256 * c:256 * (c + 1)] = yc[:, 0:256]
        outT[:, 2048 + 256 * c:2048 + 256 * (c + 1)] = yc[:, 256:512]
    return np.ascontiguousarray(outT.T).reshape(B, S, D)
